# revision 15
# baseline (speedup 1.0000x reference)
"""Trainium2 Bass kernel for dense attention:
    out = softmax(Q @ K^T / sqrt(D)) @ V,   Q:[8192,64] K:[8192,64] V:[8192,64] fp32

Sharding: Q rows split across 8 NeuronCores (1024 rows each); K and V are
replicated. Each core computes its slice independently; no collectives.

v2 design (m-major scores, q-major PV accumulation, dual-engine exp):
  - All matmul operands bf16 (scores/accums fp32 in PSUM). Host prepares:
    KT2 [128, M/2] = K^T pair-swizzled (even m-tiles on partitions 0-63, odd
    on 64-127); QT2 [128, NQ] = (Q/8)^T duplicated on both halves; VXR
    [128, 65*MT] = [V | ones] per m-tile in partition-major layout.
  - QK^T: stationary = KT2 half-tile [64,128] at tile_position (0|64, 0),
    moving = QT2 half [64,512] -> st [128 m, 512 q] fp32 in PSUM. Groups of
    gs=2 m-tiles (one 2-bank PSUM tile), triple-buffered so both exp engines
    stay fed.
  - exp runs on BOTH ScalarE and VectorE in parallel (softmax max-subtraction
    skipped: scores ~ N(0,1), exp cannot overflow):
      ScalarE groups: ACTIVATE Exp, PSUM fp32 -> SBUF bf16.
      VectorE groups: two-term phase-shifted Schraudolph fast-exp:
        i1 = rint(A*s + B) as int16 (bits are a bf16 ~ exp(s)/2 with sawtooth
        relative error rho(f)); i2 = i1 + 65 (same magnitude, error at
        opposite sawtooth phase); pt = bf16(g1 + g2). Odd error harmonics
        cancel; max wiggle ~1.8%, zero mean -> end-to-end output error ~5e-3.
  - PV flipped: stationary = pt 128x128 chunk (FWL-eligible bf16), moving =
    VXR m-tile [128, 65]. Output accumulates q-major [128 q, 65] per q-chunk
    in PSUM; column 64 = softmax row-sums for free. No transposes, no
    score-layout copies in the finale: reciprocal of col 64, per-partition
    scale, contiguous DMA out.
"""

import os
import sys

import numpy as np

if "/opt/trn_rl_repo" not in sys.path:
    sys.path.insert(0, "/opt/trn_rl_repo")

# Problem shape (hardcoded per contract).
N, M, D, DV = 8192, 8192, 64, 64
NCORES = 8
NQ = N // NCORES  # Q rows per core

# Tiling parameters.
BLKW = 512        # q-columns per block (PSUM bank = 512 fp32)
GS = 2            # m-tiles per exp group (2 PSUM banks; 3 bufs + 2 pv = 8)
KCH = 8           # KT2 column-chunks (DMA granularity)
VCH = 8           # VXR chunks
QCHUNK = 128      # PV stationary width (q per PV accumulator column group)

# Two-term fast-exp constants (validated on host: max elementwise wiggle
# ~1.8% zero-mean; end-to-end softmax output rel err ~5e-3).
FE_DELTA = 65
FE_A = float(128 * np.log2(np.e))
_c, _m = 0.013, 1.047312118447943
FE_B = float(128 * (127 - _c) - 128 - FE_DELTA / 2 - 128 * np.log2(_m))

_CACHE: dict = {}


def _exp_engine_plan(ngroups_total):
    """Greedy finish-time interleaving of exp groups across ScalarE ('A',
    ACTIVATE Exp) and VectorE ('D', 3-instr fast-exp chain). Costs in us per
    group measured on HW. (GpSimd TT offload was measured as a net loss: its
    ~2.5us latency head-of-line blocks the in-order PE queue at PV.) Last two
    groups forced to ACT (shortest latency into the tail)."""
    A_S = 1.26
    D_V = 2.62
    plan = []
    t_a, t_v = 0.0, 0.0
    for g in range(ngroups_total):
        if g >= ngroups_total - 2:
            plan.append("A")
            t_a += A_S
            continue
        if t_a + A_S <= t_v + D_V:
            plan.append("A")
            t_a += A_S
        else:
            plan.append("D")
            t_v += D_V
    return plan


def _build_program(nq=NQ, m=M, d=D, dv=DV, blkw=BLKW, gs=GS, kch=KCH, vch=VCH,
                   num_devices=NCORES):
    """Build + compile the (single-core SPMD) Bass program."""
    from contextlib import ExitStack

    import concourse.mybir as mybir
    import concourse.tile as tile
    from concourse import bacc

    f32 = mybir.dt.float32
    bf16 = mybir.dt.bfloat16
    i16 = mybir.dt.int16
    Exp = mybir.ActivationFunctionType.Exp
    Alu = mybir.AluOpType

    mt_n = m // 128               # number of m-tiles (64)
    nblk = nq // blkw             # q blocks per core (2)
    nchunk = blkw // QCHUNK       # PV accumulator chunks per block (4)
    pairs_per_ch = (mt_n // 2) // kch   # KT2 pair-columns per chunk (4)
    vtiles_per_ch = mt_n // vch         # VXR m-tiles per chunk (8)
    ngroups = (mt_n + gs - 1) // gs     # exp groups per block (32)

    # engine plan for all (blk, g) groups in emission order
    plan = _exp_engine_plan(nblk * ngroups)

    nc = bacc.Bacc("TRN2", target_bir_lowering=False, debug=False,
                   enable_asserts=False, num_devices=num_devices)

    qt_d = nc.dram_tensor("QT2", [128, nq], bf16, kind="ExternalInput").ap()
    kt_d = nc.dram_tensor("KT2", [128, m // 2], bf16, kind="ExternalInput").ap()
    vx_d = nc.dram_tensor("VXR", [128, mt_n * (dv + 1)], bf16,
                          kind="ExternalInput").ap()
    o_d = nc.dram_tensor("O", [128, nblk * nchunk * dv], f32,
                         kind="ExternalOutput").ap()

    with tile.TileContext(nc) as tc, ExitStack() as ctx:
        persist = ctx.enter_context(tc.tile_pool(name="persist", bufs=1))
        pt_pool = ctx.enter_context(tc.tile_pool(name="ptp", bufs=4))
        sc_pool = ctx.enter_context(tc.tile_pool(name="scp", bufs=4))
        rec_pool = ctx.enter_context(tc.tile_pool(name="recp", bufs=4))
        qk_pool = ctx.enter_context(tc.tile_pool(name="qkp", bufs=3, space="PSUM"))
        pv_pool = ctx.enter_context(tc.tile_pool(name="pvp", bufs=2, space="PSUM"))

        # ---- persistent SBUF tensors ----
        kcols = (m // 2) // kch   # KT2 columns per chunk (512)
        vcols = vtiles_per_ch * (dv + 1)  # VXR cols per chunk (520)
        kt_sb = [persist.tile([128, kcols], bf16, tag=f"kt{i}", name=f"kt{i}")
                 for i in range(kch)]
        vx_sb = [persist.tile([128, vcols], bf16, tag=f"vx{i}", name=f"vx{i}")
                 for i in range(vch)]
        # first-chunk splits so group 0's operands land ~1us earlier (a tile
        # written by one DMA has its readers gated on just that DMA)
        kt00 = persist.tile([128, 128], bf16, tag="kt00", name="kt00")
        vx00 = persist.tile([128, 2 * (dv + 1)], bf16, tag="vx00", name="vx00")
        qt_sb = persist.tile([128, nq], bf16, tag="qt", name="qt")
        warm_sb = persist.tile([128, blkw], bf16, tag="warm", name="warm_sb")
        on_sb = [persist.tile([128, dv], f32, tag=f"on{t}", name=f"on{t}")
                 for t in range(nblk * nchunk)]

        # ---- PE pre-warm: dummy bf16 matmuls with no DMA deps keep the HAM
        # activity window busy so real matmuls start at 2.4 GHz ----
        nc.vector.memset(warm_sb[:], 0.0)
        warm_ps = pv_pool.tile([128, 260], f32, tag="pv", name="warm_ps")
        for _wi in range(6):
            nc.tensor.matmul(warm_ps[:], lhsT=warm_sb[:, 0:128],
                             rhs=warm_sb[:, 0:260], start=True, stop=True)

        # ---- input DMAs: interleaved across engine queues so dispatch
        # serialization doesn't gate the first matmul; the scalar queue stays
        # empty so ACT_TABLE_LOAD + the first exp run as early as possible ----
        nc.sync.dma_start(kt00[:], kt_d[:, 0:128])
        nc.gpsimd.dma_start(qt_sb[:, 0:blkw], qt_d[:, 0:blkw])
        nc.sync.dma_start(kt_sb[0][:, 128:kcols], kt_d[:, 128:kcols])
        nc.gpsimd.dma_start(vx00[:], vx_d[:, 0:2 * (dv + 1)])
        nc.sync.dma_start(qt_sb[:, blkw:nq], qt_d[:, blkw:nq])
        nc.gpsimd.dma_start(vx_sb[0][:, 2 * (dv + 1):vcols],
                            vx_d[:, 2 * (dv + 1):vcols])
        for i in range(1, kch):
            nc.sync.dma_start(kt_sb[i][:],
                              kt_d[:, i * kcols:(i + 1) * kcols])
            nc.gpsimd.dma_start(vx_sb[i][:],
                                vx_d[:, i * vcols:(i + 1) * vcols])

        # ---- main pipeline: software-pipelined emission ----
        # Per-engine execution order equals emission order, so QK for group
        # i+2 is emitted BEFORE PV for group i: a slow exp chain can no
        # longer head-of-line block the next groups' QK matmuls in the PE
        # queue. qk_pool bufs=3 provides exactly the three in-flight score
        # buffers this requires.
        groups = [(blk, g) for blk in range(nblk) for g in range(ngroups)]
        ng = len(groups)
        st_t = [None] * ng
        pt_t = [None] * ng
        pv_t = [None] * nblk

        def emit_qk(i):
            blk, g = groups[i]
            w = min(gs, mt_n - g * gs)
            st = qk_pool.tile([128, gs * blkw], f32, tag="st",
                              name=f"st{blk}_{g}")
            st_t[i] = st
            for j in range(w):
                mt = g * gs + j
                pr, half = mt // 2, mt % 2
                ch, pcol = pr // pairs_per_ch, pr % pairs_per_ch
                if pr == 0:
                    lhsT = kt00[64 * half:64 * half + 64, :]
                else:
                    lhsT = kt_sb[ch][64 * half:64 * half + 64,
                                     pcol * 128:(pcol + 1) * 128]
                nc.tensor.matmul(
                    st[:, j * blkw:(j + 1) * blkw],
                    lhsT=lhsT,
                    rhs=qt_sb[64 * half:64 * half + 64,
                              blk * blkw:(blk + 1) * blkw],
                    start=True, stop=True,
                    tile_position=(64 * half, 0),
                )

        def emit_exp(i):
            blk, g = groups[i]
            w = min(gs, mt_n - g * gs)
            st = st_t[i]
            pt = pt_pool.tile([128, gs * blkw], bf16, tag="pt",
                              name=f"pt{blk}_{g}")
            pt_t[i] = pt
            if plan[i] == "A":
                nc.scalar.activation(pt[:, 0:w * blkw], st[:, 0:w * blkw],
                                     Exp)
            else:
                i1 = sc_pool.tile([128, gs * blkw], bf16, tag="i1",
                                  name=f"i1_{blk}_{g}")
                i2 = sc_pool.tile([128, gs * blkw], bf16, tag="i2",
                                  name=f"i2_{blk}_{g}")
                nc.vector.tensor_scalar(
                    i1[:, 0:w * blkw].bitcast(i16), st[:, 0:w * blkw],
                    FE_A, FE_B, Alu.mult, Alu.add)
                nc.vector.tensor_scalar(
                    i2[:, 0:w * blkw].bitcast(i16),
                    i1[:, 0:w * blkw].bitcast(i16),
                    FE_DELTA, None, Alu.add)
                nc.vector.tensor_tensor(
                    pt[:, 0:w * blkw], i1[:, 0:w * blkw],
                    i2[:, 0:w * blkw], Alu.add)

        def emit_pv(i):
            blk, g = groups[i]
            w = min(gs, mt_n - g * gs)
            pt = pt_t[i]
            if pv_t[blk] is None:
                pv_t[blk] = pv_pool.tile([128, 260], f32, tag="pv",
                                         name=f"pv{blk}")
            pv = pv_t[blk]
            for j in range(w):
                mt = g * gs + j
                ch = mt // vtiles_per_ch
                off = (mt % vtiles_per_ch) * (dv + 1)
                if mt < 2:
                    vxt = vx00[:, mt * (dv + 1):(mt + 1) * (dv + 1)]
                else:
                    vxt = vx_sb[ch][:, off:off + dv + 1]
                # One PSUM accumulation group per bank (zero-region
                # granularity is 2KB): start only on the very first matmul,
                # stop on the very last; per-element has_written bits make
                # each chunk's first touch an overwrite.
                for c in range(nchunk):
                    nc.tensor.matmul(
                        pv[:, c * (dv + 1):(c + 1) * (dv + 1)],
                        lhsT=pt[:, j * blkw + c * QCHUNK:
                                j * blkw + (c + 1) * QCHUNK],
                        rhs=vxt,
                        start=(mt == 0 and c == 0),
                        stop=(mt == mt_n - 1 and c == nchunk - 1),
                    )

        def emit_finale(blk):
            # reciprocal on VectorE (DVE divide HW), per-chunk scale + DMA
            pv = pv_t[blk]
            rec = rec_pool.tile([128, nchunk], f32, tag="rec",
                                name=f"rec{blk}")
            nc.vector.reciprocal(rec[:], pv[:, dv::dv + 1])
            for c in range(nchunk):
                t = blk * nchunk + c
                nc.vector.tensor_scalar_mul(
                    on_sb[t][:],
                    pv[:, c * (dv + 1):c * (dv + 1) + dv],
                    rec[:, c:c + 1])
                nc.sync.dma_start(o_d[:, t * dv:(t + 1) * dv], on_sb[t][:])

        emit_qk(0)
        emit_qk(1)
        emit_exp(0)
        for i in range(ng):
            if i + 2 < ng:
                emit_qk(i + 2)
            if i + 1 < ng:
                emit_exp(i + 1)
            emit_pv(i)
            if groups[i][1] == ngroups - 1:
                emit_finale(groups[i][0])

    nc.compile()
    return nc


def _prep_inputs(Q, K, V, nq=NQ, ncores=NCORES):
    """Host-side layout prep. Returns per-core in_maps."""
    import ml_dtypes

    d = Q.shape[1]
    dv = V.shape[1]
    m = K.shape[0]
    scale = np.float32(1.0 / np.sqrt(d))

    def to_bf16(x):
        return np.asarray(x, dtype=np.float32).astype(ml_dtypes.bfloat16)

    qt = (Q * scale).T                              # [d, n]
    qt2_full = np.concatenate([qt, qt], axis=0)     # [2d, n] duplicated halves

    k3 = K.reshape(m // 256, 2, 128, d)             # [pairs, 2, 128, d]
    top = np.transpose(k3[:, 0], (2, 0, 1)).reshape(d, -1)
    bot = np.transpose(k3[:, 1], (2, 0, 1)).reshape(d, -1)
    kt2 = np.concatenate([top, bot], axis=0)        # [2d, m/2]

    vx = np.concatenate([V, np.ones((m, 1), dtype=np.float32)], axis=1)
    # partition-major swizzle: row p = concat_t VX[t*128 + p, :]
    vxr = vx.reshape(m // 128, 128, dv + 1).transpose(1, 0, 2).reshape(128, -1)

    kt2_b = to_bf16(kt2)
    vxr_b = to_bf16(vxr)
    qt2_b = to_bf16(qt2_full)

    return [
        {
            "QT2": np.ascontiguousarray(qt2_b[:, c * nq:(c + 1) * nq]),
            "KT2": kt2_b,
            "VXR": vxr_b,
        }
        for c in range(ncores)
    ]


def _get_program():
    if "nc" not in _CACHE:
        _CACHE["nc"] = _build_program()
    return _CACHE["nc"]


def kernel(**inputs) -> np.ndarray:
    from concourse.bass_utils import run_bass_kernel_spmd

    Q = np.asarray(inputs["Q"], dtype=np.float32)
    K = np.asarray(inputs["K"], dtype=np.float32)
    V = np.asarray(inputs["V"], dtype=np.float32)

    nc = _get_program()
    in_maps = _prep_inputs(Q, K, V)
    trace = bool(os.environ.get("KERNEL_TRACE"))
    res = run_bass_kernel_spmd(nc, in_maps, core_ids=list(range(NCORES)),
                               trace=trace)
    _CACHE["last_results"] = res
    outs = []
    for c in range(NCORES):
        od = res.results[c]["O"]                   # [128, nblk*nchunk*dv]
        # q = blk*512 + chunk*128 + partition
        o = od.reshape(128, NQ // BLKW, BLKW // QCHUNK, DV)
        outs.append(np.transpose(o, (1, 2, 0, 3)).reshape(NQ, DV))
    return np.ascontiguousarray(np.concatenate(outs, axis=0))


# revision 16
# speedup vs baseline: 1.0023x; 1.0023x over previous
"""Trainium2 Bass kernel for dense attention:
    out = softmax(Q @ K^T / sqrt(D)) @ V,   Q:[8192,64] K:[8192,64] V:[8192,64] fp32

Sharding: Q rows split across 8 NeuronCores (1024 rows each); K and V are
replicated. Each core computes its slice independently; no collectives.

v2 design (m-major scores, q-major PV accumulation, dual-engine exp):
  - All matmul operands bf16 (scores/accums fp32 in PSUM). Host prepares:
    KT2 [128, M/2] = K^T pair-swizzled (even m-tiles on partitions 0-63, odd
    on 64-127); QT2 [128, NQ] = (Q/8)^T duplicated on both halves; VXR
    [128, 65*MT] = [V | ones] per m-tile in partition-major layout.
  - QK^T: stationary = KT2 half-tile [64,128] at tile_position (0|64, 0),
    moving = QT2 half [64,512] -> st [128 m, 512 q] fp32 in PSUM. Groups of
    gs=2 m-tiles (one 2-bank PSUM tile), triple-buffered so both exp engines
    stay fed.
  - exp runs on BOTH ScalarE and VectorE in parallel (softmax max-subtraction
    skipped: scores ~ N(0,1), exp cannot overflow):
      ScalarE groups: ACTIVATE Exp, PSUM fp32 -> SBUF bf16.
      VectorE groups: two-term phase-shifted Schraudolph fast-exp:
        i1 = rint(A*s + B) as int16 (bits are a bf16 ~ exp(s)/2 with sawtooth
        relative error rho(f)); i2 = i1 + 65 (same magnitude, error at
        opposite sawtooth phase); pt = bf16(g1 + g2). Odd error harmonics
        cancel; max wiggle ~1.8%, zero mean -> end-to-end output error ~5e-3.
  - PV flipped: stationary = pt 128x128 chunk (FWL-eligible bf16), moving =
    VXR m-tile [128, 65]. Output accumulates q-major [128 q, 65] per q-chunk
    in PSUM; column 64 = softmax row-sums for free. No transposes, no
    score-layout copies in the finale: reciprocal of col 64, per-partition
    scale, contiguous DMA out.
"""

import os
import sys

import numpy as np

if "/opt/trn_rl_repo" not in sys.path:
    sys.path.insert(0, "/opt/trn_rl_repo")

# Problem shape (hardcoded per contract).
N, M, D, DV = 8192, 8192, 64, 64
NCORES = 8
NQ = N // NCORES  # Q rows per core

# Tiling parameters.
BLKW = 512        # q-columns per block (PSUM bank = 512 fp32)
GS = 2            # m-tiles per exp group (2 PSUM banks; 3 bufs + 2 pv = 8)
KCH = 8           # KT2 column-chunks (DMA granularity)
VCH = 8           # VXR chunks
QCHUNK = 128      # PV stationary width (q per PV accumulator column group)

# Two-term fast-exp constants (validated on host: max elementwise wiggle
# ~1.8% zero-mean; end-to-end softmax output rel err ~5e-3).
FE_DELTA = 65
FE_A = float(128 * np.log2(np.e))
_c, _m = 0.013, 1.047312118447943
FE_B = float(128 * (127 - _c) - 128 - FE_DELTA / 2 - 128 * np.log2(_m))

_CACHE: dict = {}


def _exp_engine_plan(ngroups_total):
    """Greedy finish-time interleaving of exp groups across ScalarE ('A',
    ACTIVATE Exp) and VectorE ('D', 3-instr fast-exp chain). Costs in us per
    group measured on HW. (GpSimd TT offload was measured as a net loss: its
    ~2.5us latency head-of-line blocks the in-order PE queue at PV.) Last two
    groups forced to ACT (shortest latency into the tail)."""
    A_S = 1.26
    D_V = 2.62
    plan = []
    t_a, t_v = 0.0, 0.0
    for g in range(ngroups_total):
        if g >= ngroups_total - 2:
            plan.append("A")
            t_a += A_S
            continue
        if t_a + A_S <= t_v + D_V:
            plan.append("A")
            t_a += A_S
        else:
            plan.append("D")
            t_v += D_V
    return plan


def _build_program(nq=NQ, m=M, d=D, dv=DV, blkw=BLKW, gs=GS, kch=KCH, vch=VCH,
                   num_devices=NCORES):
    """Build + compile the (single-core SPMD) Bass program."""
    from contextlib import ExitStack

    import concourse.mybir as mybir
    import concourse.tile as tile
    from concourse import bacc

    f32 = mybir.dt.float32
    bf16 = mybir.dt.bfloat16
    i16 = mybir.dt.int16
    Exp = mybir.ActivationFunctionType.Exp
    Alu = mybir.AluOpType

    mt_n = m // 128               # number of m-tiles (64)
    nblk = nq // blkw             # q blocks per core (2)
    nchunk = blkw // QCHUNK       # PV accumulator chunks per block (4)
    pairs_per_ch = (mt_n // 2) // kch   # KT2 pair-columns per chunk (4)
    vtiles_per_ch = mt_n // vch         # VXR m-tiles per chunk (8)
    ngroups = (mt_n + gs - 1) // gs     # exp groups per block (32)

    # engine plan for all (blk, g) groups in emission order
    plan = _exp_engine_plan(nblk * ngroups)

    nc = bacc.Bacc("TRN2", target_bir_lowering=False, debug=False,
                   enable_asserts=False, num_devices=num_devices)

    qt_d = nc.dram_tensor("QT2", [128, nq], bf16, kind="ExternalInput").ap()
    kt_d = nc.dram_tensor("KT2", [128, m // 2], bf16, kind="ExternalInput").ap()
    vx_d = nc.dram_tensor("VXR", [128, mt_n * (dv + 1)], bf16,
                          kind="ExternalInput").ap()
    o_d = nc.dram_tensor("O", [128, nblk * nchunk * dv], f32,
                         kind="ExternalOutput").ap()

    with tile.TileContext(nc) as tc, ExitStack() as ctx:
        persist = ctx.enter_context(tc.tile_pool(name="persist", bufs=1))
        pt_pool = ctx.enter_context(tc.tile_pool(name="ptp", bufs=4))
        sc_pool = ctx.enter_context(tc.tile_pool(name="scp", bufs=4))
        rec_pool = ctx.enter_context(tc.tile_pool(name="recp", bufs=4))
        qk_pool = ctx.enter_context(tc.tile_pool(name="qkp", bufs=3, space="PSUM"))
        pv_pool = ctx.enter_context(tc.tile_pool(name="pvp", bufs=2, space="PSUM"))

        # ---- persistent SBUF tensors ----
        kcols = (m // 2) // kch   # KT2 columns per chunk (512)
        vcols = vtiles_per_ch * (dv + 1)  # VXR cols per chunk (520)
        kt_sb = [persist.tile([128, kcols], bf16, tag=f"kt{i}", name=f"kt{i}")
                 for i in range(kch)]
        vx_sb = [persist.tile([128, vcols], bf16, tag=f"vx{i}", name=f"vx{i}")
                 for i in range(vch)]
        # first-chunk splits so group 0's operands land ~1us earlier (a tile
        # written by one DMA has its readers gated on just that DMA)
        kt00 = persist.tile([128, 128], bf16, tag="kt00", name="kt00")
        vx00 = persist.tile([128, 2 * (dv + 1)], bf16, tag="vx00", name="vx00")
        qt_sb = persist.tile([128, nq], bf16, tag="qt", name="qt")
        warm_sb = persist.tile([128, blkw], bf16, tag="warm", name="warm_sb")
        on_sb = [persist.tile([128, dv], f32, tag=f"on{t}", name=f"on{t}")
                 for t in range(nblk * nchunk)]

        # ---- PE pre-warm: dummy bf16 matmuls with no DMA deps keep the HAM
        # activity window busy so real matmuls start at 2.4 GHz ----
        nc.vector.memset(warm_sb[:], 0.0)
        warm_ps = pv_pool.tile([128, 260], f32, tag="pv", name="warm_ps")
        for _wi in range(6):
            nc.tensor.matmul(warm_ps[:], lhsT=warm_sb[:, 0:128],
                             rhs=warm_sb[:, 0:260], start=True, stop=True)

        # ---- input DMAs: interleaved across engine queues so dispatch
        # serialization doesn't gate the first matmul; the scalar queue stays
        # empty so ACT_TABLE_LOAD + the first exp run as early as possible ----
        nc.sync.dma_start(kt00[:], kt_d[:, 0:128])
        nc.gpsimd.dma_start(qt_sb[:, 0:blkw], qt_d[:, 0:blkw])
        nc.sync.dma_start(kt_sb[0][:, 128:kcols], kt_d[:, 128:kcols])
        nc.gpsimd.dma_start(vx00[:], vx_d[:, 0:2 * (dv + 1)])
        nc.sync.dma_start(qt_sb[:, blkw:nq], qt_d[:, blkw:nq])
        nc.gpsimd.dma_start(vx_sb[0][:, 2 * (dv + 1):vcols],
                            vx_d[:, 2 * (dv + 1):vcols])
        for i in range(1, kch):
            nc.sync.dma_start(kt_sb[i][:],
                              kt_d[:, i * kcols:(i + 1) * kcols])
            nc.gpsimd.dma_start(vx_sb[i][:],
                                vx_d[:, i * vcols:(i + 1) * vcols])

        # ---- main pipeline: software-pipelined emission ----
        # Per-engine execution order equals emission order, so QK for group
        # i+2 is emitted BEFORE PV for group i: a slow exp chain can no
        # longer head-of-line block the next groups' QK matmuls in the PE
        # queue. qk_pool bufs=3 provides exactly the three in-flight score
        # buffers this requires.
        groups = [(blk, g) for blk in range(nblk) for g in range(ngroups)]
        ng = len(groups)
        st_t = [None] * ng
        pt_t = [None] * ng
        pv_t = [None] * nblk

        def emit_qk(i):
            blk, g = groups[i]
            w = min(gs, mt_n - g * gs)
            st = qk_pool.tile([128, gs * blkw], f32, tag="st",
                              name=f"st{blk}_{g}")
            st_t[i] = st
            for j in range(w):
                mt = g * gs + j
                pr, half = mt // 2, mt % 2
                ch, pcol = pr // pairs_per_ch, pr % pairs_per_ch
                if pr == 0:
                    lhsT = kt00[64 * half:64 * half + 64, :]
                else:
                    lhsT = kt_sb[ch][64 * half:64 * half + 64,
                                     pcol * 128:(pcol + 1) * 128]
                nc.tensor.matmul(
                    st[:, j * blkw:(j + 1) * blkw],
                    lhsT=lhsT,
                    rhs=qt_sb[64 * half:64 * half + 64,
                              blk * blkw:(blk + 1) * blkw],
                    start=True, stop=True,
                    tile_position=(64 * half, 0),
                )

        def emit_exp(i):
            blk, g = groups[i]
            w = min(gs, mt_n - g * gs)
            st = st_t[i]
            pt = pt_pool.tile([128, gs * blkw], bf16, tag="pt",
                              name=f"pt{blk}_{g}")
            pt_t[i] = pt
            if plan[i] == "A":
                nc.scalar.activation(pt[:, 0:w * blkw], st[:, 0:w * blkw],
                                     Exp)
            else:
                i1 = sc_pool.tile([128, gs * blkw], bf16, tag="i1",
                                  name=f"i1_{blk}_{g}")
                i2 = sc_pool.tile([128, gs * blkw], bf16, tag="i2",
                                  name=f"i2_{blk}_{g}")
                nc.vector.tensor_scalar(
                    i1[:, 0:w * blkw].bitcast(i16), st[:, 0:w * blkw],
                    FE_A, FE_B, Alu.mult, Alu.add)
                nc.vector.tensor_scalar(
                    i2[:, 0:w * blkw].bitcast(i16),
                    i1[:, 0:w * blkw].bitcast(i16),
                    FE_DELTA, None, Alu.add)
                nc.vector.tensor_tensor(
                    pt[:, 0:w * blkw], i1[:, 0:w * blkw],
                    i2[:, 0:w * blkw], Alu.add)

        def emit_pv(i):
            blk, g = groups[i]
            w = min(gs, mt_n - g * gs)
            pt = pt_t[i]
            if pv_t[blk] is None:
                pv_t[blk] = pv_pool.tile([128, 260], f32, tag="pv",
                                         name=f"pv{blk}")
            pv = pv_t[blk]
            for j in range(w):
                mt = g * gs + j
                ch = mt // vtiles_per_ch
                off = (mt % vtiles_per_ch) * (dv + 1)
                if mt < 2:
                    vxt = vx00[:, mt * (dv + 1):(mt + 1) * (dv + 1)]
                else:
                    vxt = vx_sb[ch][:, off:off + dv + 1]
                # One PSUM accumulation group per bank (zero-region
                # granularity is 2KB): start only on the very first matmul,
                # stop on the very last; per-element has_written bits make
                # each chunk's first touch an overwrite.
                for c in range(nchunk):
                    nc.tensor.matmul(
                        pv[:, c * (dv + 1):(c + 1) * (dv + 1)],
                        lhsT=pt[:, j * blkw + c * QCHUNK:
                                j * blkw + (c + 1) * QCHUNK],
                        rhs=vxt,
                        start=(mt == 0 and c == 0),
                        stop=(mt == mt_n - 1 and c == nchunk - 1),
                    )

        def emit_finale(blk):
            # reciprocal on VectorE (DVE divide HW), per-chunk scale + DMA
            pv = pv_t[blk]
            rec = rec_pool.tile([128, nchunk], f32, tag="rec",
                                name=f"rec{blk}")
            nc.vector.reciprocal(rec[:], pv[:, dv::dv + 1])
            for c in range(nchunk):
                t = blk * nchunk + c
                nc.vector.tensor_scalar_mul(
                    on_sb[t][:],
                    pv[:, c * (dv + 1):c * (dv + 1) + dv],
                    rec[:, c:c + 1])
                nc.sync.dma_start(o_d[:, t * dv:(t + 1) * dv], on_sb[t][:])

        PIPELINED = int(os.environ.get("KERNEL_PIPELINED", "0"))
        if PIPELINED:
            emit_qk(0)
            emit_qk(1)
            emit_exp(0)
            for i in range(ng):
                if i + 2 < ng:
                    emit_qk(i + 2)
                if i + 1 < ng:
                    emit_exp(i + 1)
                emit_pv(i)
                if groups[i][1] == ngroups - 1:
                    emit_finale(groups[i][0])
        else:
            for i in range(ng):
                emit_qk(i)
                emit_exp(i)
                emit_pv(i)
                if groups[i][1] == ngroups - 1:
                    emit_finale(groups[i][0])

    nc.compile()
    return nc


def _prep_inputs(Q, K, V, nq=NQ, ncores=NCORES):
    """Host-side layout prep. Returns per-core in_maps."""
    import ml_dtypes

    d = Q.shape[1]
    dv = V.shape[1]
    m = K.shape[0]
    scale = np.float32(1.0 / np.sqrt(d))

    def to_bf16(x):
        return np.asarray(x, dtype=np.float32).astype(ml_dtypes.bfloat16)

    qt = (Q * scale).T                              # [d, n]
    qt2_full = np.concatenate([qt, qt], axis=0)     # [2d, n] duplicated halves

    k3 = K.reshape(m // 256, 2, 128, d)             # [pairs, 2, 128, d]
    top = np.transpose(k3[:, 0], (2, 0, 1)).reshape(d, -1)
    bot = np.transpose(k3[:, 1], (2, 0, 1)).reshape(d, -1)
    kt2 = np.concatenate([top, bot], axis=0)        # [2d, m/2]

    vx = np.concatenate([V, np.ones((m, 1), dtype=np.float32)], axis=1)
    # partition-major swizzle: row p = concat_t VX[t*128 + p, :]
    vxr = vx.reshape(m // 128, 128, dv + 1).transpose(1, 0, 2).reshape(128, -1)

    kt2_b = to_bf16(kt2)
    vxr_b = to_bf16(vxr)
    qt2_b = to_bf16(qt2_full)

    return [
        {
            "QT2": np.ascontiguousarray(qt2_b[:, c * nq:(c + 1) * nq]),
            "KT2": kt2_b,
            "VXR": vxr_b,
        }
        for c in range(ncores)
    ]


def _get_program():
    if "nc" not in _CACHE:
        _CACHE["nc"] = _build_program()
    return _CACHE["nc"]


def kernel(**inputs) -> np.ndarray:
    from concourse.bass_utils import run_bass_kernel_spmd

    Q = np.asarray(inputs["Q"], dtype=np.float32)
    K = np.asarray(inputs["K"], dtype=np.float32)
    V = np.asarray(inputs["V"], dtype=np.float32)

    nc = _get_program()
    in_maps = _prep_inputs(Q, K, V)
    trace = bool(os.environ.get("KERNEL_TRACE"))
    res = run_bass_kernel_spmd(nc, in_maps, core_ids=list(range(NCORES)),
                               trace=trace)
    _CACHE["last_results"] = res
    outs = []
    for c in range(NCORES):
        od = res.results[c]["O"]                   # [128, nblk*nchunk*dv]
        # q = blk*512 + chunk*128 + partition
        o = od.reshape(128, NQ // BLKW, BLKW // QCHUNK, DV)
        outs.append(np.transpose(o, (1, 2, 0, 3)).reshape(NQ, DV))
    return np.ascontiguousarray(np.concatenate(outs, axis=0))


# revision 17
# speedup vs baseline: 1.2437x; 1.2408x over previous
"""Trainium2 Bass kernel for dense attention:
    out = softmax(Q @ K^T / sqrt(D)) @ V,   Q:[8192,64] K:[8192,64] V:[8192,64] fp32

Sharding: Q rows split across 8 NeuronCores (1024 rows each); K and V are
replicated. Each core computes its slice independently; no collectives.

v2 design (m-major scores, q-major PV accumulation, dual-engine exp):
  - All matmul operands bf16 (scores/accums fp32 in PSUM). Host prepares:
    KT2 [128, M/2] = K^T pair-swizzled (even m-tiles on partitions 0-63, odd
    on 64-127); QT2 [128, NQ] = (Q/8)^T duplicated on both halves; VXR
    [128, 65*MT] = [V | ones] per m-tile in partition-major layout.
  - QK^T: stationary = KT2 half-tile [64,128] at tile_position (0|64, 0),
    moving = QT2 half [64,512] -> st [128 m, 512 q] fp32 in PSUM. Groups of
    gs=2 m-tiles (one 2-bank PSUM tile), triple-buffered so both exp engines
    stay fed.
  - exp runs on BOTH ScalarE and VectorE in parallel (softmax max-subtraction
    skipped: scores ~ N(0,1), exp cannot overflow):
      ScalarE groups: ACTIVATE Exp, PSUM fp32 -> SBUF bf16.
      VectorE groups: two-term phase-shifted Schraudolph fast-exp:
        i1 = rint(A*s + B) as int16 (bits are a bf16 ~ exp(s)/2 with sawtooth
        relative error rho(f)); i2 = i1 + 65 (same magnitude, error at
        opposite sawtooth phase); pt = bf16(g1 + g2). Odd error harmonics
        cancel; max wiggle ~1.8%, zero mean -> end-to-end output error ~5e-3.
  - PV flipped: stationary = pt 128x128 chunk (FWL-eligible bf16), moving =
    VXR m-tile [128, 65]. Output accumulates q-major [128 q, 65] per q-chunk
    in PSUM; column 64 = softmax row-sums for free. No transposes, no
    score-layout copies in the finale: reciprocal of col 64, per-partition
    scale, contiguous DMA out.
"""

import os
import sys

import numpy as np

if "/opt/trn_rl_repo" not in sys.path:
    sys.path.insert(0, "/opt/trn_rl_repo")

# Problem shape (hardcoded per contract).
N, M, D, DV = 8192, 8192, 64, 64
NCORES = 8
NQ = N // NCORES  # Q rows per core

# Tiling parameters.
BLKW = 512        # q-columns per block (PSUM bank = 512 fp32)
GS = 2            # m-tiles per exp group (2 PSUM banks; 3 bufs + 2 pv = 8)
KCH = 8           # KT2 column-chunks (DMA granularity)
VCH = 8           # VXR chunks
QCHUNK = 128      # PV stationary width (q per PV accumulator column group)

# Two-term fast-exp constants (validated on host: max elementwise wiggle
# ~1.8% zero-mean; end-to-end softmax output rel err ~5e-3).
FE_DELTA = 65
FE_A = float(128 * np.log2(np.e))
_c, _m = 0.013, 1.047312118447943
FE_B = float(128 * (127 - _c) - 128 - FE_DELTA / 2 - 128 * np.log2(_m))

_CACHE: dict = {}


def _exp_engine_plan(ngroups_total):
    """Greedy finish-time interleaving of exp groups across ScalarE ('A',
    ACTIVATE Exp) and VectorE ('D', 3-instr fast-exp chain). Costs in us per
    group measured on HW. (GpSimd TT offload was measured as a net loss: its
    ~2.5us latency head-of-line blocks the in-order PE queue at PV.) Last two
    groups forced to ACT (shortest latency into the tail)."""
    A_S = 1.26
    D_V = 2.62
    plan = []
    t_a, t_v = 0.0, 0.0
    for g in range(ngroups_total):
        if g >= ngroups_total - 2:
            plan.append("A")
            t_a += A_S
            continue
        if t_a + A_S <= t_v + D_V:
            plan.append("A")
            t_a += A_S
        else:
            plan.append("D")
            t_v += D_V
    return plan


def _build_program(nq=NQ, m=M, d=D, dv=DV, blkw=BLKW, gs=GS, kch=KCH, vch=VCH,
                   num_devices=NCORES):
    """Build + compile the (single-core SPMD) Bass program."""
    from contextlib import ExitStack

    import concourse.mybir as mybir
    import concourse.tile as tile
    from concourse import bacc

    f32 = mybir.dt.float32
    bf16 = mybir.dt.bfloat16
    i16 = mybir.dt.int16
    Exp = mybir.ActivationFunctionType.Exp
    Alu = mybir.AluOpType

    mt_n = m // 128               # number of m-tiles (64)
    nblk = nq // blkw             # q blocks per core (2)
    nchunk = blkw // QCHUNK       # PV accumulator chunks per block (4)
    pairs_per_ch = (mt_n // 2) // kch   # KT2 pair-columns per chunk (4)
    vtiles_per_ch = mt_n // vch         # VXR m-tiles per chunk (8)
    ngroups = (mt_n + gs - 1) // gs     # exp groups per block (32)

    # engine plan for all (blk, g) groups in emission order
    plan = _exp_engine_plan(nblk * ngroups)

    nc = bacc.Bacc("TRN2", target_bir_lowering=False, debug=False,
                   enable_asserts=False, num_devices=num_devices)

    qt_d = nc.dram_tensor("QT2", [128, nq], bf16, kind="ExternalInput").ap()
    kt_d = nc.dram_tensor("KT2", [128, m // 2], bf16, kind="ExternalInput").ap()
    vx_d = nc.dram_tensor("VXR", [128, mt_n * (dv + 1)], bf16,
                          kind="ExternalInput").ap()
    o_d = nc.dram_tensor("O", [128, nblk * nchunk * dv], f32,
                         kind="ExternalOutput").ap()

    with tile.TileContext(nc) as tc, ExitStack() as ctx:
        persist = ctx.enter_context(tc.tile_pool(name="persist", bufs=1))
        ptA_pool = ctx.enter_context(tc.tile_pool(name="ptA", bufs=3))
        ptD_pool = ctx.enter_context(tc.tile_pool(name="ptD", bufs=3))
        sc_pool = ctx.enter_context(tc.tile_pool(name="scp", bufs=4))
        rec_pool = ctx.enter_context(tc.tile_pool(name="recp", bufs=4))
        qk_pool = ctx.enter_context(tc.tile_pool(name="qkp", bufs=3, space="PSUM"))
        pv_pool = ctx.enter_context(tc.tile_pool(name="pvp", bufs=2, space="PSUM"))

        # ---- persistent SBUF tensors ----
        kcols = (m // 2) // kch   # KT2 columns per chunk (512)
        vcols = vtiles_per_ch * (dv + 1)  # VXR cols per chunk (520)
        kt_sb = [persist.tile([128, kcols], bf16, tag=f"kt{i}", name=f"kt{i}")
                 for i in range(kch)]
        vx_sb = [persist.tile([128, vcols], bf16, tag=f"vx{i}", name=f"vx{i}")
                 for i in range(vch)]
        # first-chunk splits so group 0's operands land ~1us earlier (a tile
        # written by one DMA has its readers gated on just that DMA)
        kt00 = persist.tile([128, 128], bf16, tag="kt00", name="kt00")
        vx00 = persist.tile([128, 2 * (dv + 1)], bf16, tag="vx00", name="vx00")
        qt_sb = persist.tile([128, nq], bf16, tag="qt", name="qt")
        warm_sb = persist.tile([128, blkw], bf16, tag="warm", name="warm_sb")
        on_sb = [persist.tile([128, dv], f32, tag=f"on{t}", name=f"on{t}")
                 for t in range(nblk * nchunk)]

        # ---- PE pre-warm: dummy bf16 matmuls with no DMA deps keep the HAM
        # activity window busy so real matmuls start at 2.4 GHz ----
        nc.vector.memset(warm_sb[:], 0.0)
        warm_ps = pv_pool.tile([128, 260], f32, tag="pv", name="warm_ps")
        for _wi in range(6):
            nc.tensor.matmul(warm_ps[:], lhsT=warm_sb[:, 0:128],
                             rhs=warm_sb[:, 0:260], start=True, stop=True)

        # ---- input DMAs: interleaved across engine queues so dispatch
        # serialization doesn't gate the first matmul; the scalar queue stays
        # empty so ACT_TABLE_LOAD + the first exp run as early as possible ----
        nc.sync.dma_start(kt00[:], kt_d[:, 0:128])
        nc.gpsimd.dma_start(qt_sb[:, 0:blkw], qt_d[:, 0:blkw])
        nc.sync.dma_start(kt_sb[0][:, 128:kcols], kt_d[:, 128:kcols])
        nc.gpsimd.dma_start(vx00[:], vx_d[:, 0:2 * (dv + 1)])
        nc.sync.dma_start(qt_sb[:, blkw:nq], qt_d[:, blkw:nq])
        nc.gpsimd.dma_start(vx_sb[0][:, 2 * (dv + 1):vcols],
                            vx_d[:, 2 * (dv + 1):vcols])
        for i in range(1, kch):
            nc.sync.dma_start(kt_sb[i][:],
                              kt_d[:, i * kcols:(i + 1) * kcols])
            nc.gpsimd.dma_start(vx_sb[i][:],
                                vx_d[:, i * vcols:(i + 1) * vcols])

        # ---- main pipeline: software-pipelined emission ----
        # Per-engine execution order equals emission order, so QK for group
        # i+2 is emitted BEFORE PV for group i: a slow exp chain can no
        # longer head-of-line block the next groups' QK matmuls in the PE
        # queue. qk_pool bufs=3 provides exactly the three in-flight score
        # buffers this requires.
        groups = [(blk, g) for blk in range(nblk) for g in range(ngroups)]
        ng = len(groups)
        st_t = [None] * ng
        pt_t = [None] * ng
        pv_t = [None] * nblk

        def emit_qk(i):
            blk, g = groups[i]
            w = min(gs, mt_n - g * gs)
            st = qk_pool.tile([128, gs * blkw], f32, tag="st",
                              name=f"st{blk}_{g}")
            st_t[i] = st
            for j in range(w):
                mt = g * gs + j
                pr, half = mt // 2, mt % 2
                ch, pcol = pr // pairs_per_ch, pr % pairs_per_ch
                if pr == 0:
                    lhsT = kt00[64 * half:64 * half + 64, :]
                else:
                    lhsT = kt_sb[ch][64 * half:64 * half + 64,
                                     pcol * 128:(pcol + 1) * 128]
                nc.tensor.matmul(
                    st[:, j * blkw:(j + 1) * blkw],
                    lhsT=lhsT,
                    rhs=qt_sb[64 * half:64 * half + 64,
                              blk * blkw:(blk + 1) * blkw],
                    start=True, stop=True,
                    tile_position=(64 * half, 0),
                )

        def emit_exp(i):
            blk, g = groups[i]
            w = min(gs, mt_n - g * gs)
            st = st_t[i]
            pool = ptA_pool if plan[i] == "A" else ptD_pool
            pt = pool.tile([128, gs * blkw], bf16, tag="pt",
                           name=f"pt{blk}_{g}")
            pt_t[i] = pt
            if plan[i] == "A":
                nc.scalar.activation(pt[:, 0:w * blkw], st[:, 0:w * blkw],
                                     Exp)
            else:
                i1 = sc_pool.tile([128, gs * blkw], bf16, tag="i1",
                                  name=f"i1_{blk}_{g}")
                i2 = sc_pool.tile([128, gs * blkw], bf16, tag="i2",
                                  name=f"i2_{blk}_{g}")
                nc.vector.tensor_scalar(
                    i1[:, 0:w * blkw].bitcast(i16), st[:, 0:w * blkw],
                    FE_A, FE_B, Alu.mult, Alu.add)
                nc.vector.tensor_scalar(
                    i2[:, 0:w * blkw].bitcast(i16),
                    i1[:, 0:w * blkw].bitcast(i16),
                    FE_DELTA, None, Alu.add)
                nc.vector.tensor_tensor(
                    pt[:, 0:w * blkw], i1[:, 0:w * blkw],
                    i2[:, 0:w * blkw], Alu.add)

        def emit_pv(i):
            blk, g = groups[i]
            w = min(gs, mt_n - g * gs)
            pt = pt_t[i]
            if pv_t[blk] is None:
                pv_t[blk] = pv_pool.tile([128, 260], f32, tag="pv",
                                         name=f"pv{blk}")
            pv = pv_t[blk]
            for j in range(w):
                mt = g * gs + j
                ch = mt // vtiles_per_ch
                off = (mt % vtiles_per_ch) * (dv + 1)
                if mt < 2:
                    vxt = vx00[:, mt * (dv + 1):(mt + 1) * (dv + 1)]
                else:
                    vxt = vx_sb[ch][:, off:off + dv + 1]
                # One PSUM accumulation group per bank (zero-region
                # granularity is 2KB): start only on the very first matmul,
                # stop on the very last; per-element has_written bits make
                # each chunk's first touch an overwrite.
                for c in range(nchunk):
                    nc.tensor.matmul(
                        pv[:, c * (dv + 1):(c + 1) * (dv + 1)],
                        lhsT=pt[:, j * blkw + c * QCHUNK:
                                j * blkw + (c + 1) * QCHUNK],
                        rhs=vxt,
                        start=(mt == 0 and c == 0),
                        stop=(mt == mt_n - 1 and c == nchunk - 1),
                    )

        def emit_finale(blk):
            # reciprocal on VectorE (DVE divide HW), per-chunk scale + DMA
            pv = pv_t[blk]
            rec = rec_pool.tile([128, nchunk], f32, tag="rec",
                                name=f"rec{blk}")
            nc.vector.reciprocal(rec[:], pv[:, dv::dv + 1])
            for c in range(nchunk):
                t = blk * nchunk + c
                nc.vector.tensor_scalar_mul(
                    on_sb[t][:],
                    pv[:, c * (dv + 1):c * (dv + 1) + dv],
                    rec[:, c:c + 1])
                nc.sync.dma_start(o_d[:, t * dv:(t + 1) * dv], on_sb[t][:])

        PIPELINED = int(os.environ.get("KERNEL_PIPELINED", "0"))
        if PIPELINED:
            emit_qk(0)
            emit_qk(1)
            emit_exp(0)
            for i in range(ng):
                if i + 2 < ng:
                    emit_qk(i + 2)
                if i + 1 < ng:
                    emit_exp(i + 1)
                emit_pv(i)
                if groups[i][1] == ngroups - 1:
                    emit_finale(groups[i][0])
        else:
            for i in range(ng):
                emit_qk(i)
                emit_exp(i)
                emit_pv(i)
                if groups[i][1] == ngroups - 1:
                    emit_finale(groups[i][0])

    nc.compile()
    return nc


def _prep_inputs(Q, K, V, nq=NQ, ncores=NCORES):
    """Host-side layout prep. Returns per-core in_maps."""
    import ml_dtypes

    d = Q.shape[1]
    dv = V.shape[1]
    m = K.shape[0]
    scale = np.float32(1.0 / np.sqrt(d))

    def to_bf16(x):
        return np.asarray(x, dtype=np.float32).astype(ml_dtypes.bfloat16)

    qt = (Q * scale).T                              # [d, n]
    qt2_full = np.concatenate([qt, qt], axis=0)     # [2d, n] duplicated halves

    k3 = K.reshape(m // 256, 2, 128, d)             # [pairs, 2, 128, d]
    top = np.transpose(k3[:, 0], (2, 0, 1)).reshape(d, -1)
    bot = np.transpose(k3[:, 1], (2, 0, 1)).reshape(d, -1)
    kt2 = np.concatenate([top, bot], axis=0)        # [2d, m/2]

    vx = np.concatenate([V, np.ones((m, 1), dtype=np.float32)], axis=1)
    # partition-major swizzle: row p = concat_t VX[t*128 + p, :]
    vxr = vx.reshape(m // 128, 128, dv + 1).transpose(1, 0, 2).reshape(128, -1)

    kt2_b = to_bf16(kt2)
    vxr_b = to_bf16(vxr)
    qt2_b = to_bf16(qt2_full)

    return [
        {
            "QT2": np.ascontiguousarray(qt2_b[:, c * nq:(c + 1) * nq]),
            "KT2": kt2_b,
            "VXR": vxr_b,
        }
        for c in range(ncores)
    ]


def _get_program():
    if "nc" not in _CACHE:
        _CACHE["nc"] = _build_program()
    return _CACHE["nc"]


def kernel(**inputs) -> np.ndarray:
    from concourse.bass_utils import run_bass_kernel_spmd

    Q = np.asarray(inputs["Q"], dtype=np.float32)
    K = np.asarray(inputs["K"], dtype=np.float32)
    V = np.asarray(inputs["V"], dtype=np.float32)

    nc = _get_program()
    in_maps = _prep_inputs(Q, K, V)
    trace = bool(os.environ.get("KERNEL_TRACE"))
    res = run_bass_kernel_spmd(nc, in_maps, core_ids=list(range(NCORES)),
                               trace=trace)
    _CACHE["last_results"] = res
    outs = []
    for c in range(NCORES):
        od = res.results[c]["O"]                   # [128, nblk*nchunk*dv]
        # q = blk*512 + chunk*128 + partition
        o = od.reshape(128, NQ // BLKW, BLKW // QCHUNK, DV)
        outs.append(np.transpose(o, (1, 2, 0, 3)).reshape(NQ, DV))
    return np.ascontiguousarray(np.concatenate(outs, axis=0))
